# revision 11
# baseline (speedup 1.0000x reference)
"""Custom LSTM-cell kernel for Trainium2, data-parallel over batch on 8 NeuronCores.

Math (per token, elementwise over dff except the two GEMMs):
    gates = Hi @ Wh + Zi @ Wz + bias         # [tok, 4*dff], gate order I|F|O|Z
    A   = F~ + Mi
    M_t = max(A, I~)
    I_t = exp(I~ - M_t) = exp(min(-(A-I~), 0))
    F_t = exp(A  - M_t) = exp(min(A-I~, 0))
    O_t = sigmoid(O~) = 0.5*(1 + tanh(O~/2))
    Z_t = tanh(Z~)
    N_t = F_t*Ni + I_t
    C_t = (Ci*F_t + Z_t*I_t)*m + (1-m)*Ci
    H_t = O_t*(C_t/N_t)*m + (1-m)*Hi

Device layout: tokens on partitions, gate columns on the free dim. Activations are
pre-transposed on host to fp16 [dff, tok] as the stationary matmul operand;
weights are the moving operand (fp16 in, fp32 PSUM accumulate). Biases:
F-gate bias minus I-gate bias is folded into Mi on the host, the I-gate bias is
added to M_t on GPSIMD, and the O/Z biases are added by the DVE ops that drain
PSUM into the tanh inputs — no bias matmuls, so the PE runs exactly the 1536
gate matmuls. Elementwise inputs (Mi/Ci/Ni/(1-m)Hi) stream as fp16 to halve DMA
traffic; intermediates are fp16 where a 2-byte dtype buys DVE 2x/4x modes,
outputs are computed straight to fp32. Engine split per tile: DVE gets the
PSUM-drain ops plus the tensor_scalar chain, ScalarE the four transcendentals,
GPSIMD four tensor_tensor ops (M_t/FN/N_t/H_t).
"""

import numpy as np

import concourse.bass as bass
import concourse.tile as tile
import concourse.bass_utils as bass_utils
from concourse import bacc, mybir
from concourse.bass import ts, ds

B, P, D, DFF = 256, 64, 512, 1024
NCORES = 8
BL = B // NCORES          # batches per core
TOK = BL * P              # tokens per core (2048)
NT = TOK // 128           # token tiles per core (16)
KH = DFF // 128           # Hi k-tiles (8)
KZ = D // 128             # Zi k-tiles (4)
KT = KH + KZ              # total k-tiles (12)
CH = 2                    # dff column chunks of 512 per gate
CW = 512                  # chunk width

F32 = mybir.dt.float32
F16 = mybir.dt.float16
AF = mybir.ActivationFunctionType
OP = mybir.AluOpType

_CACHE = {}


def _build(repeat: int = 1, wbufs: int = KT + 4, mode: str = "full"):
    """Build + compile the per-core Bass module. Cached per config.

    mode: "full" (default) | "mm" (matmuls + weight streaming only) |
    "ew" (elementwise + io DMAs, no matmuls) — microbenchmark variants."""
    key = (repeat, wbufs, mode)
    if key in _CACHE:
        return _CACHE[key]

    nc = bacc.Bacc("TRN2", target_bir_lowering=False, debug=False,
                   num_devices=NCORES)

    hiT = nc.dram_tensor("hiT", [KH, 128, TOK], F16, kind="ExternalInput").ap()
    ziT = nc.dram_tensor("ziT", [KZ, 128, TOK], F16, kind="ExternalInput").ap()
    w = nc.dram_tensor("w", [CH, KT, 128, 4, CW], F16, kind="ExternalInput").ap()
    biasb = nc.dram_tensor("biasb", [3, CH, CW], F32, kind="ExternalInput").ap()
    mi = nc.dram_tensor("mi", [TOK, DFF], F16, kind="ExternalInput").ap()
    ci = nc.dram_tensor("ci", [TOK, DFF], F16, kind="ExternalInput").ap()
    ni = nc.dram_tensor("ni", [TOK, DFF], F16, kind="ExternalInput").ap()
    hiom = nc.dram_tensor("hiom", [TOK, DFF], F16, kind="ExternalInput").ap()
    mpk = nc.dram_tensor("mpk", [NT, 128, 3], F32, kind="ExternalInput").ap()

    ct = nc.dram_tensor("ct", [TOK, DFF], F32, kind="ExternalOutput").ap()
    mt = nc.dram_tensor("mt", [TOK, DFF], F32, kind="ExternalOutput").ap()
    ht = nc.dram_tensor("ht", [TOK, DFF], F32, kind="ExternalOutput").ap()
    nt = nc.dram_tensor("nt", [TOK, DFF], F32, kind="ExternalOutput").ap()

    with tile.TileContext(nc) as tc:
        with (
            tc.tile_pool(name="singles", bufs=1) as singles,
            tc.tile_pool(name="wpool", bufs=wbufs) as wpool,
            tc.tile_pool(name="inpool", bufs=3) as inpool,
            tc.tile_pool(name="t16", bufs=2) as t16,
            tc.tile_pool(name="t32", bufs=2) as t32,
            tc.tile_pool(name="outp", bufs=2) as outp,
            tc.tile_pool(name="ps", bufs=8, space="PSUM") as pspool,
        ):
            # Startup: interleave first-chunk weights with the activation tiles
            # so MMs for k=0 can begin ~3us in, instead of after the full
            # 12 MiB resident load.
            act_k = [None] * KT
            wk0 = [None] * KT
            bb = singles.tile([128, 3, CH, CW], F32)
            mpk_sb = singles.tile([128, NT, 3], F32)
            QTOK = TOK // 2
            for k in range(KT):
                wt = wpool.tile([128, 4, CW], F16, tag="wk", name=f"w0k{k}")
                nc.sync.dma_start(out=wt, in_=w[0, k])
                wk0[k] = wt
                if k < KH:
                    ak = singles.tile([128, TOK], F16, name=f"hiTk{k}")
                    nc.sync.dma_start(out=ak[:, 0:QTOK], in_=hiT[k, :, 0:QTOK])
                else:
                    ak = singles.tile([128, TOK], F16, name=f"ziTk{k}")
                    nc.sync.dma_start(out=ak[:, 0:QTOK],
                                      in_=ziT[k - KH, :, 0:QTOK])
                act_k[k] = ak
                if k == 0:
                    nc.sync.dma_start(out=mpk_sb,
                                      in_=mpk.rearrange("t p c -> p t c"))
                    # bias rows broadcast to all partitions: bI | 0.5*bO | bZ
                    for r in range(3):
                        for cj in range(CH):
                            bsl = biasb[r, cj]
                            bcast = bass.AP(tensor=bsl.tensor, offset=bsl.offset,
                                            ap=[[0, 128]] + list(bsl.ap))
                            nc.gpsimd.dma_start(out=bb[:, r, cj], in_=bcast)
            for q in range(1, 2):
                for k in range(KT):
                    qs = ds(q * QTOK, QTOK)
                    src = hiT[k, :, qs] if k < KH else ziT[k - KH, :, qs]
                    nc.sync.dma_start(out=act_k[k][:, qs], in_=src)

            def emit_unit(wk, t, c, col_off, cw):
                rows = ts(t, 128)
                cols = ds(col_off, cw)
                csl = ds(col_off - c * CW, cw)   # slice within the w chunk
                if mode.startswith("mm"):
                    mw = int(mode[2:]) if len(mode) > 2 else cw
                    ps = [pspool.tile([128, cw], F32, tag="ps", name=f"ps{g}")
                          for g in range(4)]
                    for k in range(KT):
                        for g in range(4):
                            for j in range(cw // mw):
                                nc.tensor.matmul(
                                    ps[g][:, ds(j * mw, mw)], act_k[k][:, rows],
                                    wk[k][:, g, ds(col_off - c * CW + j * mw, mw)],
                                    start=(k == 0), stop=(k == KT - 1))
                    return
                mi_t = inpool.tile([128, cw], F16, tag="mi")
                nc.sync.dma_start(out=mi_t, in_=mi[rows, cols])
                ci_t = inpool.tile([128, cw], F16, tag="ci")
                nc.sync.dma_start(out=ci_t, in_=ci[rows, cols])
                ni_t = inpool.tile([128, cw], F16, tag="ni")
                nc.sync.dma_start(out=ni_t, in_=ni[rows, cols])
                ho_t = inpool.tile([128, cw], F16, tag="ho")
                nc.sync.dma_start(out=ho_t, in_=hiom[rows, cols])
                m_ap = mpk_sb[:, t, 0:1]
                om_ap = mpk_sb[:, t, 1:2]
                hm_ap = mpk_sb[:, t, 2:3]
                bsl = [bb[:, r, c, ds(col_off - c * CW, cw)] for r in range(3)]

                ps = [pspool.tile([128, cw], F32, tag="ps", name=f"ps{g}")
                      for g in range(4)]
                if mode != "ew":
                    for k in range(KT):
                        for g in range(4):
                            nc.tensor.matmul(ps[g], act_k[k][:, rows],
                                             wk[k][:, g, csl],
                                             start=(k == 0), stop=(k == KT - 1))

                psI, psF, psO, psZ = ps
                # PSUM drains first so banks free for the next tile
                A = t32.tile([128, cw], F32, tag="A")
                nc.vector.tensor_add(A, psF, mi_t)
                Dd = t16.tile([128, cw], F16, tag="Dd")
                nc.vector.tensor_sub(Dd, A, psI)
                mx = t32.tile([128, cw], F32, tag="mx")
                nc.vector.tensor_max(mx, A, psI)
                oin = t16.tile([128, cw], F16, tag="oin")
                nc.vector.scalar_tensor_tensor(oin, psO, 0.5, bsl[1],
                                               OP.mult, OP.add)
                zin = t16.tile([128, cw], F16, tag="zin")
                nc.vector.scalar_tensor_tensor(zin, psZ, 1.0, bsl[2],
                                               OP.mult, OP.add)
                Mt = outp.tile([128, cw], F32, tag="Mt")
                nc.gpsimd.tensor_add(Mt, mx, bsl[0])
                nc.sync.dma_start(out=mt[rows, cols], in_=Mt)

                p_ = t16.tile([128, cw], F16, tag="p")
                nc.vector.tensor_scalar_min(p_, Dd, 0.0)
                pn = t16.tile([128, cw], F16, tag="pn")
                nc.vector.tensor_scalar(pn, Dd, -1.0, 0.0, OP.mult, OP.min)
                Ft = t16.tile([128, cw], F16, tag="Ft")
                nc.scalar.activation(Ft, p_, AF.Exp)
                It = t16.tile([128, cw], F16, tag="It")
                nc.scalar.activation(It, pn, AF.Exp)
                th = t16.tile([128, cw], F16, tag="th")
                nc.scalar.activation(th, oin, AF.Tanh)
                Zt = t16.tile([128, cw], F16, tag="Zt")
                nc.scalar.activation(Zt, zin, AF.Tanh)

                FN = t16.tile([128, cw], F16, tag="FN")
                nc.gpsimd.tensor_mul(FN, Ft, ni_t)
                Nt = outp.tile([128, cw], F32, tag="Nt")
                nc.gpsimd.tensor_add(Nt, FN, It)
                nc.sync.dma_start(out=nt[rows, cols], in_=Nt)
                rec = t32.tile([128, cw], F32, tag="rec")
                nc.vector.reciprocal_approx_fast(rec, Nt)
                rec2 = t32.tile([128, cw], F32, tag="rec2")
                nc.vector.tensor_scalar_mul(rec2, rec, hm_ap)
                mF = t16.tile([128, cw], F16, tag="mF")
                nc.vector.tensor_scalar(mF, Ft, m_ap, om_ap, OP.mult, OP.add)
                p1 = t16.tile([128, cw], F16, tag="p1")
                nc.vector.tensor_mul(p1, ci_t, mF)
                t2 = t16.tile([128, cw], F16, tag="t2")
                nc.vector.tensor_mul(t2, Zt, It)
                Ct = outp.tile([128, cw], F32, tag="Ct")
                nc.vector.scalar_tensor_tensor(Ct, t2, m_ap, p1,
                                               OP.mult, OP.add)
                nc.sync.dma_start(out=ct[rows, cols], in_=Ct)
                R = t32.tile([128, cw], F32, tag="R")
                nc.vector.tensor_mul(R, Ct, rec2)
                u = t32.tile([128, cw], F32, tag="u")
                nc.vector.scalar_tensor_tensor(u, th, 1.0, R,
                                               OP.add, OP.mult)
                Ht = outp.tile([128, cw], F32, tag="Ht")
                nc.gpsimd.tensor_add(Ht, u, ho_t)
                nc.sync.dma_start(out=ht[rows, cols], in_=Ht)

            for rep in range(repeat):
                for c in range(CH):
                    if c == 0 and rep == 0:
                        wk = wk0
                    else:
                        wk = []
                        for k in range(KT):
                            wt = wpool.tile([128, 4, CW], F16, tag="wk")
                            nc.sync.dma_start(out=wt, in_=w[c, k])
                            wk.append(wt)
                    last = (rep == repeat - 1 and c == CH - 1)
                    for t in range(NT):
                        if last and t == NT - 1:
                            # the final tile's elementwise chain is the drain
                            # tail of the whole kernel; finish on a narrow
                            # piece so the serial chain after the last matmul
                            # is short
                            emit_unit(wk, t, c, c * CW, 3 * CW // 4)
                            emit_unit(wk, t, c, c * CW + 3 * CW // 4, CW // 4)
                        else:
                            emit_unit(wk, t, c, c * CW, CW)

    nc.compile()
    _CACHE[key] = nc
    return nc


def _prep_inputs(inputs):
    """Host-side shard + reformat. Returns per-core input maps."""
    f32, f16 = np.float32, np.float16
    g = {k: np.asarray(v) for k, v in inputs.items()}

    Wh = np.concatenate([g['WI_w'], g['WF_w'], g['WO_w'], g['WZ_w']], axis=1)
    Wz = np.concatenate([g['RI_w'], g['RF_w'], g['RO_w'], g['RZ_w']], axis=1)
    bias = np.concatenate([g['WI_b'] + g['RI_b'], g['WF_b'] + g['RF_b'],
                           g['WO_b'] + g['RO_b'], g['WZ_b'] + g['RZ_b']])
    Wcat = np.vstack([Wh, Wz]).astype(f16)                       # [1536, 4096]
    w_l = np.ascontiguousarray(
        Wcat.reshape(KT, 128, 4, CH, CW).transpose(3, 0, 1, 2, 4))
    bI, bF, bO, bZ = bias.reshape(4, DFF).astype(f32)
    biasb_l = np.ascontiguousarray(
        np.stack([bI, 0.5 * bO, bZ]).reshape(3, CH, CW))
    mi_shift = (bF - bI)[None, :]                # folded into Mi on host

    in_maps = []
    for c in range(NCORES):
        sl = slice(c * BL, (c + 1) * BL)
        Hi_c = g['Hi'][sl].reshape(TOK, DFF)
        Zi_c = g['Zi'][sl].reshape(TOK, D)
        m_c = g['m'][sl].reshape(TOK, 1).astype(f32)
        hiT = np.ascontiguousarray(Hi_c.T).astype(f16).reshape(KH, 128, TOK)
        ziT = np.ascontiguousarray(Zi_c.T).astype(f16).reshape(KZ, 128, TOK)
        mpk = np.concatenate([m_c, 1.0 - m_c, 0.5 * m_c],
                             axis=1).astype(f32).reshape(NT, 128, 3)
        in_maps.append({
            "hiT": hiT,
            "ziT": ziT,
            "w": w_l,
            "biasb": biasb_l,
            "mi": (g['Mi'][sl].reshape(TOK, DFF) + mi_shift).astype(f16),
            "ci": g['Ci'][sl].reshape(TOK, DFF).astype(f16),
            "ni": g['Ni'][sl].reshape(TOK, DFF).astype(f16),
            "hiom": ((1.0 - m_c) * Hi_c).astype(f16),
            "mpk": mpk,
        })
    return in_maps


def _gather(results):
    def cat(name):
        full = np.concatenate(
            [results[c][name].reshape(BL, P, DFF) for c in range(NCORES)],
            axis=0)
        return np.ascontiguousarray(full, dtype=np.float32)
    return cat("ct"), cat("mt"), cat("ht"), cat("nt")


def kernel(**inputs):
    nc = _build(repeat=1)
    in_maps = _prep_inputs(inputs)
    res = bass_utils.run_bass_kernel_spmd(nc, in_maps,
                                          core_ids=list(range(NCORES)))
    return _gather(res.results)


# revision 13
# speedup vs baseline: 1.9326x; 1.9326x over previous
"""Custom LSTM-cell kernel for Trainium2, data-parallel over batch on 8 NeuronCores.

Math (per token, elementwise over dff except the two GEMMs):
    gates = Hi @ Wh + Zi @ Wz + bias         # [tok, 4*dff], gate order I|F|O|Z
    A   = F~ + Mi
    M_t = max(A, I~)
    I_t = exp(I~ - M_t) = exp(min(-(A-I~), 0))
    F_t = exp(A  - M_t) = exp(min(A-I~, 0))
    O_t = sigmoid(O~) = 0.5*(1 + tanh(O~/2))
    Z_t = tanh(Z~)
    N_t = F_t*Ni + I_t
    C_t = (Ci*F_t + Z_t*I_t)*m + (1-m)*Ci
    H_t = O_t*(C_t/N_t)*m + (1-m)*Hi

Device layout: tokens on partitions, gate columns on the free dim. Activations are
pre-transposed on host to fp16 [dff, tok] as the stationary matmul operand;
weights are the moving operand (fp16 in, fp32 PSUM accumulate). Biases:
F-gate bias minus I-gate bias is folded into Mi on the host, the I-gate bias is
added to M_t on GPSIMD, and the O/Z biases are added by the DVE ops that drain
PSUM into the tanh inputs — no bias matmuls, so the PE runs exactly the 1536
gate matmuls. Elementwise inputs (Mi/Ci/Ni/(1-m)Hi) stream as fp16 to halve DMA
traffic; intermediates are fp16 where a 2-byte dtype buys DVE 2x/4x modes,
outputs are computed straight to fp32. Engine split per tile: DVE gets the
PSUM-drain ops plus the tensor_scalar chain, ScalarE the four transcendentals,
GPSIMD four tensor_tensor ops (M_t/FN/N_t/H_t).
"""

import numpy as np

import concourse.bass as bass
import concourse.tile as tile
import concourse.bass_utils as bass_utils
from concourse import bacc, mybir
from concourse.bass import ts, ds

B, P, D, DFF = 256, 64, 512, 1024
NCORES = 8
BL = B // NCORES          # batches per core
TOK = BL * P              # tokens per core (2048)
NT = TOK // 128           # token tiles per core (16)
KH = DFF // 128           # Hi k-tiles (8)
KZ = D // 128             # Zi k-tiles (4)
KT = KH + KZ              # total k-tiles (12)
CH = 2                    # dff column chunks of 512 per gate
CW = 512                  # chunk width

F32 = mybir.dt.float32
F16 = mybir.dt.float16
AF = mybir.ActivationFunctionType
OP = mybir.AluOpType

_CACHE = {}


def _build(repeat: int = 1, wbufs: int = KT + 4, mode: str = "full", mw: int = 0):
    """Build + compile the per-core Bass module. Cached per config.

    mode: "full" (default) | "mm" (matmuls + weight streaming only) |
    "ew" (elementwise + io DMAs, no matmuls) — microbenchmark variants."""
    key = (repeat, wbufs, mode, mw)
    if key in _CACHE:
        return _CACHE[key]

    nc = bacc.Bacc("TRN2", target_bir_lowering=False, debug=False,
                   num_devices=NCORES)

    hiT = nc.dram_tensor("hiT", [KH, 128, TOK], F16, kind="ExternalInput").ap()
    ziT = nc.dram_tensor("ziT", [KZ, 128, TOK], F16, kind="ExternalInput").ap()
    w = nc.dram_tensor("w", [CH, KT, 128, 4, CW], F16, kind="ExternalInput").ap()
    biasb = nc.dram_tensor("biasb", [3, CH, CW], F32, kind="ExternalInput").ap()
    mi = nc.dram_tensor("mi", [TOK, DFF], F16, kind="ExternalInput").ap()
    ci = nc.dram_tensor("ci", [TOK, DFF], F16, kind="ExternalInput").ap()
    ni = nc.dram_tensor("ni", [TOK, DFF], F16, kind="ExternalInput").ap()
    hiom = nc.dram_tensor("hiom", [TOK, DFF], F16, kind="ExternalInput").ap()
    mpk = nc.dram_tensor("mpk", [NT, 128, 3], F32, kind="ExternalInput").ap()

    ct = nc.dram_tensor("ct", [TOK, DFF], F32, kind="ExternalOutput").ap()
    mt = nc.dram_tensor("mt", [TOK, DFF], F32, kind="ExternalOutput").ap()
    ht = nc.dram_tensor("ht", [TOK, DFF], F32, kind="ExternalOutput").ap()
    nt = nc.dram_tensor("nt", [TOK, DFF], F32, kind="ExternalOutput").ap()

    with tile.TileContext(nc) as tc:
        with (
            tc.tile_pool(name="singles", bufs=1) as singles,
            tc.tile_pool(name="wpool", bufs=wbufs) as wpool,
            tc.tile_pool(name="inpool", bufs=3) as inpool,
            tc.tile_pool(name="t16", bufs=2) as t16,
            tc.tile_pool(name="t32", bufs=2) as t32,
            tc.tile_pool(name="outp", bufs=2) as outp,
            tc.tile_pool(name="ps", bufs=8, space="PSUM") as pspool,
        ):
            # Startup: interleave first-chunk weights with the activation tiles
            # so MMs for k=0 can begin ~3us in, instead of after the full
            # 12 MiB resident load.
            act_k = [None] * KT
            wk0 = [None] * KT
            bb = singles.tile([128, 3, CH, CW], F32)
            mpk_sb = singles.tile([128, NT, 3], F32)
            QTOK = TOK // 2
            for k in range(KT):
                wt = wpool.tile([128, 4, CW], F16, tag="wk", name=f"w0k{k}")
                nc.sync.dma_start(out=wt, in_=w[0, k])
                wk0[k] = wt
                if k < KH:
                    ak = singles.tile([128, TOK], F16, name=f"hiTk{k}")
                    nc.sync.dma_start(out=ak[:, 0:QTOK], in_=hiT[k, :, 0:QTOK])
                else:
                    ak = singles.tile([128, TOK], F16, name=f"ziTk{k}")
                    nc.sync.dma_start(out=ak[:, 0:QTOK],
                                      in_=ziT[k - KH, :, 0:QTOK])
                act_k[k] = ak
                if k == 0:
                    nc.sync.dma_start(out=mpk_sb,
                                      in_=mpk.rearrange("t p c -> p t c"))
                    # bias rows broadcast to all partitions: bI | 0.5*bO | bZ
                    for r in range(3):
                        for cj in range(CH):
                            bsl = biasb[r, cj]
                            bcast = bass.AP(tensor=bsl.tensor, offset=bsl.offset,
                                            ap=[[0, 128]] + list(bsl.ap))
                            nc.gpsimd.dma_start(out=bb[:, r, cj], in_=bcast)
            for q in range(1, 2):
                for k in range(KT):
                    qs = ds(q * QTOK, QTOK)
                    src = hiT[k, :, qs] if k < KH else ziT[k - KH, :, qs]
                    nc.sync.dma_start(out=act_k[k][:, qs], in_=src)

            def emit_unit(wk, t, c, col_off, cw):
                rows = ts(t, 128)
                cols = ds(col_off, cw)
                csl = ds(col_off - c * CW, cw)   # slice within the w chunk
                if mode.startswith("mm"):
                    mmw = int(mode[2:]) if len(mode) > 2 else cw
                    ps = [pspool.tile([128, cw], F32, tag="ps", name=f"ps{g}")
                          for g in range(4)]
                    for k in range(KT):
                        for g in range(4):
                            for j in range(cw // mmw):
                                nc.tensor.matmul(
                                    ps[g][:, ds(j * mmw, mmw)], act_k[k][:, rows],
                                    wk[k][:, g, ds(col_off - c * CW + j * mmw, mmw)],
                                    start=(k == 0), stop=(k == KT - 1))
                    return
                mi_t = inpool.tile([128, cw], F16, tag="mi")
                nc.sync.dma_start(out=mi_t, in_=mi[rows, cols])
                ci_t = inpool.tile([128, cw], F16, tag="ci")
                nc.sync.dma_start(out=ci_t, in_=ci[rows, cols])
                ni_t = inpool.tile([128, cw], F16, tag="ni")
                nc.sync.dma_start(out=ni_t, in_=ni[rows, cols])
                ho_t = inpool.tile([128, cw], F16, tag="ho")
                nc.sync.dma_start(out=ho_t, in_=hiom[rows, cols])
                m_ap = mpk_sb[:, t, 0:1]
                om_ap = mpk_sb[:, t, 1:2]
                hm_ap = mpk_sb[:, t, 2:3]
                bsl = [bb[:, r, c, ds(col_off - c * CW, cw)] for r in range(3)]

                ps = [pspool.tile([128, cw], F32, tag="ps", name=f"ps{g}")
                      for g in range(4)]
                if mode != "ew":
                    mw_ = mw or cw
                    for k in range(KT):
                        for g in range(4):
                            for j in range(0, cw, mw_):
                                jw = min(mw_, cw - j)
                                nc.tensor.matmul(
                                    ps[g][:, ds(j, jw)], act_k[k][:, rows],
                                    wk[k][:, g, ds(col_off - c * CW + j, jw)],
                                    start=(k == 0), stop=(k == KT - 1))

                psI, psF, psO, psZ = ps
                # PSUM drains first so banks free for the next tile
                A = t32.tile([128, cw], F32, tag="A")
                nc.vector.tensor_add(A, psF, mi_t)
                Dd = t16.tile([128, cw], F16, tag="Dd")
                nc.vector.tensor_sub(Dd, A, psI)
                mx = t32.tile([128, cw], F32, tag="mx")
                nc.vector.tensor_max(mx, A, psI)
                oin = t16.tile([128, cw], F16, tag="oin")
                nc.vector.scalar_tensor_tensor(oin, psO, 0.5, bsl[1],
                                               OP.mult, OP.add)
                zin = t16.tile([128, cw], F16, tag="zin")
                nc.vector.scalar_tensor_tensor(zin, psZ, 1.0, bsl[2],
                                               OP.mult, OP.add)
                Mt = outp.tile([128, cw], F32, tag="Mt")
                nc.gpsimd.tensor_add(Mt, mx, bsl[0])
                nc.sync.dma_start(out=mt[rows, cols], in_=Mt)

                p_ = t16.tile([128, cw], F16, tag="p")
                nc.vector.tensor_scalar_min(p_, Dd, 0.0)
                pn = t16.tile([128, cw], F16, tag="pn")
                nc.vector.tensor_scalar(pn, Dd, -1.0, 0.0, OP.mult, OP.min)
                Ft = t16.tile([128, cw], F16, tag="Ft")
                nc.scalar.activation(Ft, p_, AF.Exp)
                It = t16.tile([128, cw], F16, tag="It")
                nc.scalar.activation(It, pn, AF.Exp)
                th = t16.tile([128, cw], F16, tag="th")
                nc.scalar.activation(th, oin, AF.Tanh)
                Zt = t16.tile([128, cw], F16, tag="Zt")
                nc.scalar.activation(Zt, zin, AF.Tanh)

                FN = t16.tile([128, cw], F16, tag="FN")
                nc.gpsimd.tensor_mul(FN, Ft, ni_t)
                Nt = outp.tile([128, cw], F32, tag="Nt")
                nc.gpsimd.tensor_add(Nt, FN, It)
                nc.sync.dma_start(out=nt[rows, cols], in_=Nt)
                rec = t32.tile([128, cw], F32, tag="rec")
                nc.vector.reciprocal_approx_fast(rec, Nt)
                rec2 = t32.tile([128, cw], F32, tag="rec2")
                nc.vector.tensor_scalar_mul(rec2, rec, hm_ap)
                mF = t16.tile([128, cw], F16, tag="mF")
                nc.vector.tensor_scalar(mF, Ft, m_ap, om_ap, OP.mult, OP.add)
                p1 = t16.tile([128, cw], F16, tag="p1")
                nc.vector.tensor_mul(p1, ci_t, mF)
                t2 = t16.tile([128, cw], F16, tag="t2")
                nc.vector.tensor_mul(t2, Zt, It)
                Ct = outp.tile([128, cw], F32, tag="Ct")
                nc.vector.scalar_tensor_tensor(Ct, t2, m_ap, p1,
                                               OP.mult, OP.add)
                nc.sync.dma_start(out=ct[rows, cols], in_=Ct)
                R = t32.tile([128, cw], F32, tag="R")
                nc.vector.tensor_mul(R, Ct, rec2)
                u = t32.tile([128, cw], F32, tag="u")
                nc.vector.scalar_tensor_tensor(u, th, 1.0, R,
                                               OP.add, OP.mult)
                Ht = outp.tile([128, cw], F32, tag="Ht")
                nc.gpsimd.tensor_add(Ht, u, ho_t)
                nc.sync.dma_start(out=ht[rows, cols], in_=Ht)

            for rep in range(repeat):
                for c in range(CH):
                    if c == 0 and rep == 0:
                        wk = wk0
                    else:
                        wk = []
                        for k in range(KT):
                            wt = wpool.tile([128, 4, CW], F16, tag="wk")
                            nc.sync.dma_start(out=wt, in_=w[c, k])
                            wk.append(wt)
                    last = (rep == repeat - 1 and c == CH - 1)
                    for t in range(NT):
                        if last and t == NT - 1:
                            # the final tile's elementwise chain is the drain
                            # tail of the whole kernel; finish on a narrow
                            # piece so the serial chain after the last matmul
                            # is short
                            emit_unit(wk, t, c, c * CW, 3 * CW // 4)
                            emit_unit(wk, t, c, c * CW + 3 * CW // 4, CW // 4)
                        else:
                            emit_unit(wk, t, c, c * CW, CW)

    nc.compile()
    _CACHE[key] = nc
    return nc


def _prep_inputs(inputs):
    """Host-side shard + reformat. Returns per-core input maps."""
    f32, f16 = np.float32, np.float16
    g = {k: np.asarray(v) for k, v in inputs.items()}

    Wh = np.concatenate([g['WI_w'], g['WF_w'], g['WO_w'], g['WZ_w']], axis=1)
    Wz = np.concatenate([g['RI_w'], g['RF_w'], g['RO_w'], g['RZ_w']], axis=1)
    bias = np.concatenate([g['WI_b'] + g['RI_b'], g['WF_b'] + g['RF_b'],
                           g['WO_b'] + g['RO_b'], g['WZ_b'] + g['RZ_b']])
    Wcat = np.vstack([Wh, Wz]).astype(f16)                       # [1536, 4096]
    w_l = np.ascontiguousarray(
        Wcat.reshape(KT, 128, 4, CH, CW).transpose(3, 0, 1, 2, 4))
    bI, bF, bO, bZ = bias.reshape(4, DFF).astype(f32)
    biasb_l = np.ascontiguousarray(
        np.stack([bI, 0.5 * bO, bZ]).reshape(3, CH, CW))
    mi_shift = (bF - bI)[None, :]                # folded into Mi on host

    in_maps = []
    for c in range(NCORES):
        sl = slice(c * BL, (c + 1) * BL)
        Hi_c = g['Hi'][sl].reshape(TOK, DFF)
        Zi_c = g['Zi'][sl].reshape(TOK, D)
        m_c = g['m'][sl].reshape(TOK, 1).astype(f32)
        hiT = np.ascontiguousarray(Hi_c.T).astype(f16).reshape(KH, 128, TOK)
        ziT = np.ascontiguousarray(Zi_c.T).astype(f16).reshape(KZ, 128, TOK)
        mpk = np.concatenate([m_c, 1.0 - m_c, 0.5 * m_c],
                             axis=1).astype(f32).reshape(NT, 128, 3)
        in_maps.append({
            "hiT": hiT,
            "ziT": ziT,
            "w": w_l,
            "biasb": biasb_l,
            "mi": (g['Mi'][sl].reshape(TOK, DFF) + mi_shift).astype(f16),
            "ci": g['Ci'][sl].reshape(TOK, DFF).astype(f16),
            "ni": g['Ni'][sl].reshape(TOK, DFF).astype(f16),
            "hiom": ((1.0 - m_c) * Hi_c).astype(f16),
            "mpk": mpk,
        })
    return in_maps


def _gather(results):
    def cat(name):
        full = np.concatenate(
            [results[c][name].reshape(BL, P, DFF) for c in range(NCORES)],
            axis=0)
        return np.ascontiguousarray(full, dtype=np.float32)
    return cat("ct"), cat("mt"), cat("ht"), cat("nt")


def kernel(**inputs):
    nc = _build(repeat=1)
    in_maps = _prep_inputs(inputs)
    res = bass_utils.run_bass_kernel_spmd(nc, in_maps,
                                          core_ids=list(range(NCORES)))
    return _gather(res.results)
